# revision 46
# baseline (speedup 1.0000x reference)
"""Multi-head attention (B=4, L=2048, D=1024, H=16, hd=64) on 8 Trainium2 cores.

Sharding: 8-way tensor parallel over heads. Core c owns heads (2c, 2c+1) for
all batches: it projects qkv for its heads (x replicated, w_qkv column-sliced),
runs attention, and computes a partial out-projection with its w_out row-slice.
The host sums the 8 partials (row-parallel unshard).

Final design notes (from NTFF trace analysis; 614us baseline -> ~495us):
- fp32r matmuls at N=512 already stream at ~225ns (1 col/cycle warm); bf16
  gains nothing on the PE and costs +220ns/exp on ACT writes. So matmul
  operands stay fp32r except stage A's x/w_qkv (bf16 halves the 32MB x DMA).
- The j-loop is gated by ScalarE exp ([128,1024] ACTIVATE = ~1.11us each,
  256 total = ~285us floor). Everything else must fit in its shadow.
- S^T matmuls contract over K=64 only. Processing both heads per j as
  adjacent matmuls at base partitions 0/64 puts them in distinct PE row
  groups, so they co-execute (~2x). Packing both heads' scores for a
  512-query block into one [128, 2, 512] PSUM tile keeps exp at N=1024
  while fitting PSUM: S 2bufs x 2banks + po 2x1 + scratch 2 = 8 banks.
- Softmax 1/denom: DVE reciprocal on a [1,512] row is ~4us (single lane).
  Instead DMA-spread the denominator row to [128,4] (4 elem/lane), DVE
  reciprocal there (~60ns), DMA-gather back to a row, then the usual
  ones-matmul partition broadcast + DVE multiply, all deferred off the
  exp critical path.
- HAM: the PE clock gate drops to 1.2GHz after ~3.4us of low activity; the
  baseline lost ~90us in its last batch (no filler left -> PE sparse ->
  half clock). Filler (stage A of later batches, out-projection, norm
  tails) is paced against the *global* remaining j-iterations so work
  remains to keep the PE dense through the final batch.
- Startup: eager stage A for batch 0 computes only q(tm0)+k(all) before the
  j-loop starts; v projection/transposes and q(tm1..3) drain as early
  filler. First exp issues ~25us earlier than with full eager stage A.
"""
import os
from collections import deque
import numpy as np
from contextlib import ExitStack

B, L, D = 4, 2048, 1024
NH, HD = 16, 64
T = B * L  # 8192 tokens
NCORES = 8
TM = 512  # stage-A token macro-tile
IM = 512  # stage-B query block


def _round_fp32r(a: np.ndarray) -> np.ndarray:
    """Round fp32 to fp32r (e8m11: fp32 with low 12 mantissa bits zero), RNE."""
    u = np.ascontiguousarray(a, dtype=np.float32).view(np.uint32).copy()
    add = np.uint32(0x7FF) + ((u >> np.uint32(12)) & np.uint32(1))
    u = (u + add) & np.uint32(0xFFFFF000)
    return u.view(np.float32)


def _to_bf16(a: np.ndarray):
    import ml_dtypes

    return np.asarray(a, dtype=np.float32).astype(ml_dtypes.bfloat16)


def _build_program():
    import concourse.bacc as bacc
    import concourse.tile as tile
    from concourse import mybir

    F32 = mybir.dt.float32
    F32R = mybir.dt.float32r
    BF16 = mybir.dt.bfloat16
    EXP = mybir.ActivationFunctionType.Exp

    nc = bacc.Bacc(
        "TRN2", target_bir_lowering=False, debug=False, num_devices=NCORES
    )
    xT_d = nc.dram_tensor("xT", [D, T], BF16, kind="ExternalInput")
    wqkv_d = nc.dram_tensor("wqkv", [D, 384], BF16, kind="ExternalInput")
    wout_d = nc.dram_tensor("wout", [128, D], F32R, kind="ExternalInput")
    ones_d = nc.dram_tensor("ones", [128, 64], F32R, kind="ExternalInput")
    ident_d = nc.dram_tensor("ident", [128, 128], F32, kind="ExternalInput")
    y_d = nc.dram_tensor("y", [T, D], F32, kind="ExternalOutput")

    xT_v = xT_d.ap().rearrange("(k p) t -> p k t", p=128)  # [128, 8, T]
    wqkv_v = wqkv_d.ap().rearrange("(k p) c -> p k c", p=128)  # [128, 8, 384]

    NTM = L // TM  # stage-A macro tiles per batch
    NJ = L // 128  # key tiles per batch
    NIM = L // IM  # query blocks per batch

    with tile.TileContext(nc) as tc, ExitStack() as ctx:
        consts = ctx.enter_context(tc.tile_pool(name="consts", bufs=1))
        sb_x = ctx.enter_context(tc.tile_pool(name="sb_x", bufs=4))
        sb_qk = ctx.enter_context(tc.tile_pool(name="sb_qk", bufs=2))
        sb_v = ctx.enter_context(tc.tile_pool(name="sb_v", bufs=2))
        sb_vst = ctx.enter_context(tc.tile_pool(name="sb_vst", bufs=5))
        sb_p = ctx.enter_context(tc.tile_pool(name="sb_p", bufs=3))
        sb_o = ctx.enter_context(tc.tile_pool(name="sb_o", bufs=4))
        sb_d = ctx.enter_context(tc.tile_pool(name="sb_d", bufs=4))
        sb_oT = ctx.enter_context(tc.tile_pool(name="sb_oT", bufs=10))
        sb_y = ctx.enter_context(tc.tile_pool(name="sb_y", bufs=4))
        ps_s = ctx.enter_context(tc.tile_pool(name="ps_s", bufs=2, space="PSUM"))
        ps_po = ctx.enter_context(tc.tile_pool(name="ps_po", bufs=1, space="PSUM"))
        ps_m = ctx.enter_context(tc.tile_pool(name="ps_m", bufs=2, space="PSUM"))

        wq_t = consts.tile([128, 8, 384], BF16, tag="wqkv")
        nc.sync.dma_start(wq_t[:], wqkv_v[:])
        wo_t = consts.tile([128, D], F32R, tag="wout")
        nc.sync.dma_start(wo_t[:], wout_d[:])
        ones_t = consts.tile([128, 64], F32R, tag="ones")
        nc.sync.dma_start(ones_t[:], ones_d[:])
        ident_t = consts.tile([128, 128], F32, tag="ident")
        nc.sync.dma_start(ident_t[:], ident_d[:])

        qk_tiles = {}  # b -> (qT, kT, v_aug)

        def stage_a_units(b, split_first=False):
            """Emitter closures for batch b's qkv projection.

            Returns (eager_units, filler_units). With split_first, the eager
            part is just q(tm0) + k(all tms) - the minimum for the first
            query block's S matmuls - and everything else goes to filler.
            """
            qT_b = sb_qk.tile([128, L], F32R, tag="qT")
            kT_b = sb_qk.tile([128, L], F32R, tag="kT")
            v_b = sb_v.tile([128, 2, NJ, 65], F32R, tag="v")
            qk_tiles[b] = (qT_b, kT_b, v_b)
            xt_tiles = {}
            vst_tiles = {}

            def ones_col():
                nc.vector.tensor_copy(
                    v_b[:, :, :, 64:65],
                    ones_t[:, 0 : 2 * NJ].rearrange(
                        "p (h j o) -> p h j o", h=2, o=1
                    ),
                )

            def xt_load(tm):
                if tm not in xt_tiles:
                    xt = sb_x.tile([128, 8, TM], BF16, tag="xt")
                    t0 = b * L + tm * TM
                    nc.sync.dma_start(xt[:], xT_v[:, :, t0 : t0 + TM])
                    xt_tiles[tm] = xt

            def col_group(tm, c):
                xt_load(tm)
                xt = xt_tiles[tm]
                psA = ps_m.tile([128, TM], mybir.dt.float32, tag="m")
                for k in range(8):
                    nc.tensor.matmul(
                        psA[:],
                        wq_t[:, k, c * 128 : (c + 1) * 128],
                        xt[:, k, :],
                        start=(k == 0),
                        stop=(k == 7),
                    )
                if c == 0:
                    nc.vector.tensor_copy(qT_b[:, tm * TM : (tm + 1) * TM], psA[:])
                elif c == 1:
                    nc.vector.tensor_copy(kT_b[:, tm * TM : (tm + 1) * TM], psA[:])
                else:
                    vst = sb_vst.tile([128, TM], mybir.dt.float32, tag="vst")
                    nc.vector.tensor_copy(vst[:], psA[:])
                    vst_tiles[tm] = vst

            def transposes(tm):
                vst = vst_tiles[tm]
                for tb in range(TM // 128):
                    jt = tm * (TM // 128) + tb
                    ptr = ps_m.tile([128, 128], mybir.dt.float32, tag="m")
                    nc.tensor.transpose(
                        ptr[:], vst[:, tb * 128 : (tb + 1) * 128], ident_t[:]
                    )
                    nc.vector.tensor_copy(v_b[:, 0, jt, 0:64], ptr[:, 0:64])
                    nc.vector.tensor_copy(v_b[:, 1, jt, 0:64], ptr[:, 64:128])

            if split_first:
                # eager: everything the first PV iteration needs (q tm0,
                # all k, v tm0); the rest carries a hard deadline (global
                # j-iteration index) enforced by pop_filler, because Tile
                # preserves emission order - emitting a consumer before its
                # producer is a silent stale read, not a stall.
                # eager: everything the first PV iteration needs (q tm0,
                # all k, v tm0); the rest carries a hard deadline (global
                # j-iteration index) enforced by pop_filler, because Tile
                # preserves emission order - emitting a consumer before its
                # producer is a silent stale read, not a stall.
                eager = [(1.0, lambda: col_group(0, 0))]
                eager += [
                    (0.0, lambda tm=tm: xt_load(tm)) for tm in range(1, NTM)
                ]
                eager += [
                    (1.0, lambda tm=tm: col_group(tm, 1)) for tm in range(NTM)
                ]
                eager.append((1.0, lambda: col_group(0, 2)))
                eager.append((0.7, lambda: transposes(0)))
                eager.append((0.1, ones_col))
                fill = []
                for tm in range(1, NTM):
                    # v/tr(tm) feed PV j=4*tm (emitted at iter 4*tm+1);
                    # q(tm) feeds S of im=tm (emitted at iter 16*tm).
                    fill.append(
                        (1.0, lambda tm=tm: col_group(tm, 2), 4 * tm - 1)
                    )
                    fill.append((0.7, lambda tm=tm: transposes(tm), 4 * tm))
                    fill.append(
                        (1.0, lambda tm=tm: col_group(tm, 0), 16 * tm - 2)
                    )
                return eager, fill
            fill = [(0.1, ones_col, None), (0.0, lambda: xt_load(0), None)]
            for tm in range(NTM):
                # prefetch the next tm's x tile so its DMA overlaps this
                # tm's matmuls instead of stalling them
                if tm + 1 < NTM:
                    fill.append((0.0, lambda tm=tm: xt_load(tm + 1), None))
                for c in range(3):
                    fill.append(
                        (1.0, lambda tm=tm, c=c: col_group(tm, c), None)
                    )
                fill.append((0.7, lambda tm=tm: transposes(tm), None))
            return [], fill

        # Three filler queues with different pacing/deadlines:
        # - tails: norm broadcast+multiply; run promptly (~1/iter) because
        #   they release pool slots (o_ev/recr/pbc) that in-order engines
        #   would otherwise deadlock on.
        # - stagea: next batch's qkv projection; must finish by its batch
        #   start, paced over the current batch's iterations.
        # - projq: out-projection tiles; paced over the *global* remaining
        #   iterations so PE work remains through the final batch (HAM).
        tails = deque()
        stagea = deque()
        projq = deque()

        drain_mode = {"on": False}
        pending_norm = []
        pending_pv = []  # final PV pair of an im, emitted after the next
        # im's S pair so the next exp is not queued behind it on the PE
        pace = {
            "s_credit": 0.0,
            "s_left": 1,
            "p_credit": 0.0,
            # finish the proj queue ~12 iterations early so the kernel
            # drain (which runs PE-sparse and HAM-throttled) stays short
            "g_left": B * NIM * NJ - 12,
            "it": 0,  # global j-iteration counter (deadline clock)
        }

        def pop_filler():
            if tails:
                tails.popleft()()
            total = sum(c for c, _, _ in stagea)
            rate = total / max(pace["s_left"], 1)
            pace["s_credit"] += rate
            while stagea and (
                pace["s_credit"] >= stagea[0][0] * 0.5
                or (stagea[0][2] is not None and stagea[0][2] <= pace["it"])
            ):
                c, fn, _dl = stagea.popleft()
                pace["s_credit"] = max(pace["s_credit"] - c, 0.0)
                fn()
            pace["s_left"] = max(pace["s_left"] - 1, 1)
            total = sum(c for c, _ in projq)
            rate = total / max(pace["g_left"], 1)
            pace["p_credit"] += rate
            while projq and pace["p_credit"] >= projq[0][0] * 0.5:
                c, fn = projq.popleft()
                pace["p_credit"] -= c
                fn()
            pace["g_left"] = max(pace["g_left"] - 1, 1)
            pace["it"] += 1

        def stage_b(b):
            qT_b, kT_b, v_b = qk_tiles[b]
            for im in range(NIM):
                i0 = im * IM
                oT_b = sb_oT.tile([128, IM], F32R, tag="oT")
                po = [
                    ps_po.tile(
                        [65, IM], mybir.dt.float32, tag=f"po{h}", name=f"po{h}"
                    )
                    for h in range(2)
                ]
                p_prev = None
                for j in range(NJ):
                    ps = ps_s.tile([128, 2, IM], mybir.dt.float32, tag="s")
                    for h in range(2):
                        hb = h * 64
                        nc.tensor.matmul(
                            ps[:, h, :],
                            kT_b[hb : hb + 64, j * 128 : (j + 1) * 128],
                            qT_b[hb : hb + 64, i0 : i0 + IM],
                            start=True,
                            stop=True,
                        )
                    if j == 0:
                        for fn in pending_pv:
                            fn()
                        pending_pv.clear()
                        for fn in pending_norm:
                            fn()
                        pending_norm.clear()
                    p_t = sb_p.tile([128, 2, IM], F32R, tag="p")
                    nc.scalar.activation(p_t[:], ps[:], EXP, scale=0.125)
                    if p_prev is not None:
                        jp, pp = p_prev
                        for h in range(2):
                            nc.tensor.matmul(
                                po[h][:],
                                v_b[:, h, jp, :],
                                pp[:, h, :],
                                start=(jp == 0),
                                stop=False,
                            )
                    p_prev = (j, p_t)
                    pop_filler()
                jp, pp = p_prev

                def pv_final(po=po, v_b=v_b, jp=jp, pp=pp):
                    for h in range(2):
                        nc.tensor.matmul(
                            po[h][:],
                            v_b[:, h, jp, :],
                            pp[:, h, :],
                            start=False,
                            stop=True,
                        )

                pending_pv.append(pv_final)

                def norm_d(po=po, oT_b=oT_b, im=im, b=b):
                    # Fast part at flush: evacuate po (releases the PSUM
                    # bank) and start the 1/denom chain. The denominator
                    # row is DMA-spread across partitions so the DVE
                    # reciprocal runs ~60ns instead of ~4us single-lane.
                    evs = []
                    for h in range(2):
                        o_ev = sb_o.tile([65, IM], mybir.dt.float32, tag="oe")
                        den_sp = sb_d.tile(
                            [128, IM // 128], mybir.dt.float32, tag="dsp"
                        )
                        rec_sp = sb_d.tile([128, IM // 128], F32R, tag="rsp")
                        recr = sb_d.tile([65, IM], F32R, tag="recr")
                        nc.vector.tensor_copy(o_ev[:], po[h][:])
                        nc.sync.dma_start(
                            den_sp[:],
                            o_ev[64:65, :].rearrange("o (g i) -> o g i", g=128),
                        )
                        with nc.allow_low_precision(reason="fp32r recip"):
                            nc.vector.reciprocal(rec_sp[:], den_sp[:])
                        nc.sync.dma_start(
                            recr[64:65, :].rearrange("o (g i) -> o g i", g=128),
                            rec_sp[:],
                        )
                        evs.append((o_ev, recr))

                    def tail():
                        with nc.allow_low_precision(reason="fp32r"):
                            for h in range(2):
                                o_ev, recr = evs[h]
                                hb = h * 64
                                pbc = ps_m.tile(
                                    [64, IM], mybir.dt.float32, tag="m"
                                )
                                nc.tensor.matmul(
                                    pbc[:],
                                    ones_t[64:65, :],
                                    recr[64:65, :],
                                    start=True,
                                    stop=True,
                                )
                                nc.vector.tensor_mul(
                                    oT_b[hb : hb + 64, :],
                                    o_ev[0:64, :],
                                    pbc[:],
                                )

                    tails.append(tail)
                    for ts in range(IM // 128):
                        projq.append(
                            (0.6, lambda ts=ts: proj_tile(ts, b, im, oT_b))
                        )

                pending_norm.append(norm_d)

                def proj_tile(ts, b=b, im=im, oT_b=oT_b):
                    y_t = sb_y.tile([128, D], mybir.dt.float32, tag="y")
                    for nh in range(2):
                        psC = ps_m.tile([128, 512], mybir.dt.float32, tag="m")
                        nc.tensor.matmul(
                            psC[:],
                            oT_b[:, ts * 128 : (ts + 1) * 128],
                            wo_t[:, nh * 512 : (nh + 1) * 512],
                            start=True,
                            stop=True,
                        )
                        if drain_mode["on"]:
                            # kernel drain: ACT is idle, let the scheduler
                            # spread evac copies across ACT+DVE instead of
                            # serializing the psC slot round-trip on DVE
                            nc.any.tensor_copy(
                                y_t[:, nh * 512 : (nh + 1) * 512], psC[:]
                            )
                        else:
                            nc.vector.tensor_copy(
                                y_t[:, nh * 512 : (nh + 1) * 512], psC[:]
                            )
                    t0 = b * L + im * IM + ts * 128
                    nc.sync.dma_start(y_d[t0 : t0 + 128, :], y_t[:])

        # batch 0: minimal eager stage A (q tm0 + k), rest through filler
        eager, fill0 = stage_a_units(0, split_first=True)
        for _c, u in eager:
            u()
        stagea.extend(fill0)
        for b in range(B):
            if b + 1 < B:
                stagea.extend(stage_a_units(b + 1)[1])
            # batch 0 drains its own residue early (v tiles are needed by
            # the first PV matmuls) plus batch 1's units; later batches
            # spread the next batch's stage A over their full span.
            pace["s_left"] = 40 if b == 0 else NIM * NJ
            stage_b(b)
        drain_mode["on"] = True
        for fn in pending_pv:
            fn()
        pending_pv.clear()
        for fn in pending_norm:
            fn()
        pending_norm.clear()
        while tails:
            tails.popleft()()
        while stagea:
            stagea.popleft()[1]()
        while projq:
            projq.popleft()[1]()
        while tails:
            tails.popleft()()
        assert not pending_norm and not stagea and not projq

    nc.compile()
    return nc


_PROGRAM = None
_LAST_EXEC_NS = None
_LAST_RESULT = None


def _get_program():
    global _PROGRAM
    if _PROGRAM is None:
        _PROGRAM = _build_program()
    return _PROGRAM


def kernel(x, mask, w_qkv, w_out):
    x = np.asarray(x)
    mask = np.asarray(mask)
    w_qkv = np.asarray(w_qkv)
    w_out = np.asarray(w_out)
    if not mask.all():
        return _masked_fallback(x, mask, w_qkv, w_out)

    from concourse.bass_utils import run_bass_kernel_spmd

    xT = _to_bf16(x.reshape(T, D).T)
    w4 = np.asarray(w_qkv, dtype=np.float32).reshape(D, 3, NH, HD)
    ones = np.ones((128, 64), dtype=np.float32)
    ident = np.eye(128, dtype=np.float32)
    in_maps = []
    for c in range(NCORES):
        hsel = [2 * c, 2 * c + 1]
        wc = _to_bf16(w4[:, :, hsel, :].reshape(D, 384))
        woc = _round_fp32r(w_out[2 * c * HD : (2 * c + 2) * HD, :])
        in_maps.append(
            {"xT": xT, "wqkv": wc, "wout": woc, "ones": ones, "ident": ident}
        )

    nc = _get_program()
    trace = os.environ.get("BASS_KERNEL_TRACE") == "1"
    res = run_bass_kernel_spmd(nc, in_maps, list(range(NCORES)), trace=trace)
    global _LAST_EXEC_NS, _LAST_RESULT
    _LAST_RESULT = res
    _LAST_EXEC_NS = getattr(res, "exec_time_ns", None)
    y = res.results[0]["y"].astype(np.float64)
    for c in range(1, NCORES):
        y += res.results[c]["y"]
    return y.astype(np.float32).reshape(B, L, D)


def _masked_fallback(x, mask, w_qkv, w_out):
    """Reference path for non-all-true masks (never hit for the spec inputs)."""
    b, l, d = x.shape
    scale = HD ** -0.5
    qkv = x.reshape(b * l, d) @ w_qkv
    qkv = qkv.reshape(b, l, 3, NH, HD).transpose(2, 0, 3, 1, 4)
    q, k, v = qkv[0], qkv[1], qkv[2]
    attn = np.einsum("bhnd,bhmd->bhnm", q, k) * scale
    attn = np.where(mask[:, None, :, :], attn, -np.inf)
    attn = attn - attn.max(axis=-1, keepdims=True)
    np.exp(attn, out=attn)
    attn /= attn.sum(axis=-1, keepdims=True)
    out = np.einsum("bhnm,bhmd->bhnd", attn, v)
    out = out.transpose(0, 2, 1, 3).reshape(b, l, d)
    return (out @ w_out).astype(np.float32)


# revision 51
# speedup vs baseline: 1.1316x; 1.1316x over previous
"""Multi-head attention (B=4, L=2048, D=1024, H=16, hd=64) on 8 Trainium2 cores.

Sharding: 8-way tensor parallel over heads. Core c owns heads (2c, 2c+1) for
all batches: it projects qkv for its heads (x replicated, w_qkv column-sliced),
runs attention, and computes a partial out-projection with its w_out row-slice.
The host sums the 8 partials (row-parallel unshard).

Final design notes (from NTFF trace analysis; 614us baseline -> ~495us):
- fp32r matmuls at N=512 already stream at ~225ns (1 col/cycle warm); bf16
  gains nothing on the PE and costs +220ns/exp on ACT writes. So matmul
  operands stay fp32r except stage A's x/w_qkv (bf16 halves the 32MB x DMA).
- The j-loop is gated by ScalarE exp ([128,1024] ACTIVATE = ~1.11us each,
  256 total = ~285us floor). Everything else must fit in its shadow.
- S^T matmuls contract over K=64 only. Processing both heads per j as
  adjacent matmuls at base partitions 0/64 puts them in distinct PE row
  groups, so they co-execute (~2x). Packing both heads' scores for a
  512-query block into one [128, 2, 512] PSUM tile keeps exp at N=1024
  while fitting PSUM: S 2bufs x 2banks + po 2x1 + scratch 2 = 8 banks.
- Softmax 1/denom: DVE reciprocal on a [1,512] row is ~4us (single lane).
  Instead DMA-spread the denominator row to [128,4] (4 elem/lane), DVE
  reciprocal there (~60ns), DMA-gather back to a row, then the usual
  ones-matmul partition broadcast + DVE multiply, all deferred off the
  exp critical path.
- HAM: the PE clock gate drops to 1.2GHz after ~3.4us of low activity; the
  baseline lost ~90us in its last batch (no filler left -> PE sparse ->
  half clock). Filler (stage A of later batches, out-projection, norm
  tails) is paced against the *global* remaining j-iterations so work
  remains to keep the PE dense through the final batch.
- Startup: eager stage A for batch 0 computes only q(tm0)+k(all) before the
  j-loop starts; v projection/transposes and q(tm1..3) drain as early
  filler. First exp issues ~25us earlier than with full eager stage A.
"""
import os
from collections import deque
import numpy as np
from contextlib import ExitStack

B, L, D = 4, 2048, 1024
NH, HD = 16, 64
T = B * L  # 8192 tokens
NCORES = 8
TM = 512  # stage-A token macro-tile
IM = 512  # stage-B query block


def _round_fp32r(a: np.ndarray) -> np.ndarray:
    """Round fp32 to fp32r (e8m11: fp32 with low 12 mantissa bits zero), RNE."""
    u = np.ascontiguousarray(a, dtype=np.float32).view(np.uint32).copy()
    add = np.uint32(0x7FF) + ((u >> np.uint32(12)) & np.uint32(1))
    u = (u + add) & np.uint32(0xFFFFF000)
    return u.view(np.float32)


def _to_bf16(a: np.ndarray):
    import ml_dtypes

    return np.asarray(a, dtype=np.float32).astype(ml_dtypes.bfloat16)


def _build_program():
    import concourse.bacc as bacc
    import concourse.tile as tile
    from concourse import mybir

    F32 = mybir.dt.float32
    F32R = mybir.dt.float32r
    BF16 = mybir.dt.bfloat16
    EXP = mybir.ActivationFunctionType.Exp

    nc = bacc.Bacc(
        "TRN2", target_bir_lowering=False, debug=False, num_devices=NCORES
    )
    xT_d = nc.dram_tensor("xT", [D, T], BF16, kind="ExternalInput")
    wqkv_d = nc.dram_tensor("wqkv", [D, 384], BF16, kind="ExternalInput")
    wout_d = nc.dram_tensor("wout", [128, D], F32R, kind="ExternalInput")
    ones_d = nc.dram_tensor("ones", [128, 64], F32R, kind="ExternalInput")
    ident_d = nc.dram_tensor("ident", [128, 128], F32R, kind="ExternalInput")
    y_d = nc.dram_tensor("y", [T, D], F32, kind="ExternalOutput")

    xT_v = xT_d.ap().rearrange("(k p) t -> p k t", p=128)  # [128, 8, T]
    wqkv_v = wqkv_d.ap().rearrange("(k p) c -> p k c", p=128)  # [128, 8, 384]

    NTM = L // TM  # stage-A macro tiles per batch
    NJ = L // 128  # key tiles per batch
    NIM = L // IM  # query blocks per batch

    with tile.TileContext(nc) as tc, ExitStack() as ctx:
        consts = ctx.enter_context(tc.tile_pool(name="consts", bufs=1))
        sb_x = ctx.enter_context(tc.tile_pool(name="sb_x", bufs=4))
        sb_qk = ctx.enter_context(tc.tile_pool(name="sb_qk", bufs=2))
        sb_v = ctx.enter_context(tc.tile_pool(name="sb_v", bufs=2))
        sb_vst = ctx.enter_context(tc.tile_pool(name="sb_vst", bufs=5))
        sb_p = ctx.enter_context(tc.tile_pool(name="sb_p", bufs=3))
        sb_o = ctx.enter_context(tc.tile_pool(name="sb_o", bufs=4))
        sb_d = ctx.enter_context(tc.tile_pool(name="sb_d", bufs=4))
        sb_oT = ctx.enter_context(tc.tile_pool(name="sb_oT", bufs=10))
        sb_y = ctx.enter_context(tc.tile_pool(name="sb_y", bufs=4))
        ps_s = ctx.enter_context(tc.tile_pool(name="ps_s", bufs=2, space="PSUM"))
        ps_po = ctx.enter_context(tc.tile_pool(name="ps_po", bufs=1, space="PSUM"))
        ps_m = ctx.enter_context(tc.tile_pool(name="ps_m", bufs=2, space="PSUM"))

        wq_t = consts.tile([128, 8, 384], BF16, tag="wqkv")
        nc.sync.dma_start(wq_t[:], wqkv_v[:])
        wo_t = consts.tile([128, D], F32R, tag="wout")
        nc.sync.dma_start(wo_t[:], wout_d[:])
        ones_t = consts.tile([128, 64], F32R, tag="ones")
        nc.sync.dma_start(ones_t[:], ones_d[:])
        ident_t = consts.tile([128, 128], F32R, tag="ident")
        nc.sync.dma_start(ident_t[:], ident_d[:])

        qk_tiles = {}  # b -> (qT, kT, v_aug)

        def stage_a_units(b, split_first=False):
            """Emitter closures for batch b's qkv projection.

            Returns (eager_units, filler_units). With split_first, the eager
            part is just q(tm0) + k(all tms) - the minimum for the first
            query block's S matmuls - and everything else goes to filler.
            """
            qT_b = sb_qk.tile([128, L], F32R, tag="qT")
            kT_b = sb_qk.tile([128, L], F32R, tag="kT")
            v_b = sb_v.tile([128, 2, NJ, 65], F32R, tag="v")
            qk_tiles[b] = (qT_b, kT_b, v_b)
            xt_tiles = {}
            vst_tiles = {}

            def ones_col():
                nc.vector.tensor_copy(
                    v_b[:, :, :, 64:65],
                    ones_t[:, 0 : 2 * NJ].rearrange(
                        "p (h j o) -> p h j o", h=2, o=1
                    ),
                )

            def xt_load(tm):
                if tm not in xt_tiles:
                    xt = sb_x.tile([128, 8, TM], BF16, tag="xt")
                    t0 = b * L + tm * TM
                    nc.sync.dma_start(xt[:], xT_v[:, :, t0 : t0 + TM])
                    xt_tiles[tm] = xt

            def col_group(tm, c):
                xt_load(tm)
                xt = xt_tiles[tm]
                psA = ps_m.tile([128, TM], mybir.dt.float32, tag="m")
                for k in range(8):
                    nc.tensor.matmul(
                        psA[:],
                        wq_t[:, k, c * 128 : (c + 1) * 128],
                        xt[:, k, :],
                        start=(k == 0),
                        stop=(k == 7),
                    )
                if c == 0:
                    nc.vector.tensor_copy(qT_b[:, tm * TM : (tm + 1) * TM], psA[:])
                elif c == 1:
                    nc.vector.tensor_copy(kT_b[:, tm * TM : (tm + 1) * TM], psA[:])
                else:
                    vst = sb_vst.tile([128, TM], F32R, tag="vst")
                    nc.vector.tensor_copy(vst[:], psA[:])
                    vst_tiles[tm] = vst

            def transposes(tm):
                vst = vst_tiles[tm]
                for tb in range(TM // 128):
                    jt = tm * (TM // 128) + tb
                    ptr = ps_m.tile([128, 128], F32R, tag="m")
                    nc.tensor.transpose(
                        ptr[:], vst[:, tb * 128 : (tb + 1) * 128], ident_t[:]
                    )
                    nc.vector.tensor_copy(v_b[:, 0, jt, 0:64], ptr[:, 0:64])
                    nc.vector.tensor_copy(v_b[:, 1, jt, 0:64], ptr[:, 64:128])

            if split_first:
                # eager: everything the first PV iteration needs (q tm0,
                # all k, v tm0); the rest carries a hard deadline (global
                # j-iteration index) enforced by pop_filler, because Tile
                # preserves emission order - emitting a consumer before its
                # producer is a silent stale read, not a stall.
                # eager: everything the first PV iteration needs (q tm0,
                # all k, v tm0); the rest carries a hard deadline (global
                # j-iteration index) enforced by pop_filler, because Tile
                # preserves emission order - emitting a consumer before its
                # producer is a silent stale read, not a stall.
                eager = [(1.0, lambda: col_group(0, 0))]
                eager += [
                    (0.0, lambda tm=tm: xt_load(tm)) for tm in range(1, NTM)
                ]
                eager += [
                    (1.0, lambda tm=tm: col_group(tm, 1)) for tm in range(NTM)
                ]
                eager.append((1.0, lambda: col_group(0, 2)))
                eager.append((0.7, lambda: transposes(0)))
                eager.append((0.1, ones_col))
                fill = []
                for tm in range(1, NTM):
                    # v/tr(tm) feed PV j=4*tm (emitted at iter 4*tm+1);
                    # q(tm) feeds S of im=tm (emitted at iter 16*tm).
                    fill.append(
                        (1.0, lambda tm=tm: col_group(tm, 2), 4 * tm - 1)
                    )
                    fill.append((0.7, lambda tm=tm: transposes(tm), 4 * tm))
                    fill.append(
                        (1.0, lambda tm=tm: col_group(tm, 0), 16 * tm - 2)
                    )
                return eager, fill
            fill = [(0.1, ones_col, None), (0.0, lambda: xt_load(0), None)]
            for tm in range(NTM):
                # prefetch the next tm's x tile so its DMA overlaps this
                # tm's matmuls instead of stalling them
                if tm + 1 < NTM:
                    fill.append((0.0, lambda tm=tm: xt_load(tm + 1), None))
                for c in range(3):
                    fill.append(
                        (1.0, lambda tm=tm, c=c: col_group(tm, c), None)
                    )
                fill.append((0.7, lambda tm=tm: transposes(tm), None))
            return [], fill

        # Three filler queues with different pacing/deadlines:
        # - tails: norm broadcast+multiply; run promptly (~1/iter) because
        #   they release pool slots (o_ev/recr/pbc) that in-order engines
        #   would otherwise deadlock on.
        # - stagea: next batch's qkv projection; must finish by its batch
        #   start, paced over the current batch's iterations.
        # - projq: out-projection tiles; paced over the *global* remaining
        #   iterations so PE work remains through the final batch (HAM).
        tails = deque()
        stagea = deque()
        projq = deque()

        drain_mode = {"on": False}
        pending_norm = []
        pending_pv = []  # final PV pair of an im, emitted after the next
        # im's S pair so the next exp is not queued behind it on the PE
        pace = {
            "s_credit": 0.0,
            "s_left": 1,
            "p_credit": 0.0,
            # finish the proj queue ~12 iterations early so the kernel
            # drain (which runs PE-sparse and HAM-throttled) stays short
            "g_left": B * NIM * NJ - 12,
            "it": 0,  # global j-iteration counter (deadline clock)
        }

        def pop_filler():
            if tails:
                tails.popleft()()
            total = sum(c for c, _, _ in stagea)
            rate = total / max(pace["s_left"], 1)
            pace["s_credit"] += rate
            while stagea and (
                pace["s_credit"] >= stagea[0][0] * 0.5
                or (stagea[0][2] is not None and stagea[0][2] <= pace["it"])
            ):
                c, fn, _dl = stagea.popleft()
                pace["s_credit"] = max(pace["s_credit"] - c, 0.0)
                fn()
            pace["s_left"] = max(pace["s_left"] - 1, 1)
            total = sum(c for c, _ in projq)
            rate = total / max(pace["g_left"], 1)
            pace["p_credit"] += rate
            while projq and pace["p_credit"] >= projq[0][0] * 0.5:
                c, fn = projq.popleft()
                pace["p_credit"] -= c
                fn()
            pace["g_left"] = max(pace["g_left"] - 1, 1)
            pace["it"] += 1

        def stage_b(b):
            qT_b, kT_b, v_b = qk_tiles[b]
            for im in range(NIM):
                i0 = im * IM
                oT_b = sb_oT.tile([128, IM], F32R, tag="oT")
                po = [
                    ps_po.tile(
                        [65, IM], mybir.dt.float32, tag=f"po{h}", name=f"po{h}"
                    )
                    for h in range(2)
                ]
                p_prev = None
                for j in range(NJ):
                    ps = ps_s.tile([128, 2, IM], mybir.dt.float32, tag="s")
                    for h in range(2):
                        hb = h * 64
                        nc.tensor.matmul(
                            ps[:, h, :],
                            kT_b[hb : hb + 64, j * 128 : (j + 1) * 128],
                            qT_b[hb : hb + 64, i0 : i0 + IM],
                            start=True,
                            stop=True,
                        )
                    if j == 0:
                        for fn in pending_pv:
                            fn()
                        pending_pv.clear()
                        for fn in pending_norm:
                            fn()
                        pending_norm.clear()
                    p_t = sb_p.tile([128, 2, IM], F32R, tag="p")
                    nc.scalar.activation(p_t[:], ps[:], EXP, scale=0.125)
                    if p_prev is not None:
                        jp, pp = p_prev
                        for h in range(2):
                            nc.tensor.matmul(
                                po[h][:],
                                v_b[:, h, jp, :],
                                pp[:, h, :],
                                start=(jp == 0),
                                stop=False,
                            )
                    p_prev = (j, p_t)
                    pop_filler()
                jp, pp = p_prev

                def pv_final(po=po, v_b=v_b, jp=jp, pp=pp):
                    for h in range(2):
                        nc.tensor.matmul(
                            po[h][:],
                            v_b[:, h, jp, :],
                            pp[:, h, :],
                            start=False,
                            stop=True,
                        )

                pending_pv.append(pv_final)

                def norm_d(po=po, oT_b=oT_b, im=im, b=b):
                    # Fast part at flush: evacuate po (releases the PSUM
                    # bank) and start the 1/denom chain. The denominator
                    # row is DMA-spread across partitions so the DVE
                    # reciprocal runs ~60ns instead of ~4us single-lane.
                    evs = []
                    for h in range(2):
                        o_ev = sb_o.tile([65, IM], mybir.dt.float32, tag="oe")
                        den_sp = sb_d.tile(
                            [128, IM // 128], mybir.dt.float32, tag="dsp"
                        )
                        rec_sp = sb_d.tile([128, IM // 128], F32R, tag="rsp")
                        recr = sb_d.tile([65, IM], F32R, tag="recr")
                        nc.vector.tensor_copy(o_ev[:], po[h][:])
                        nc.sync.dma_start(
                            den_sp[:],
                            o_ev[64:65, :].rearrange("o (g i) -> o g i", g=128),
                        )
                        with nc.allow_low_precision(reason="fp32r recip"):
                            nc.vector.reciprocal(rec_sp[:], den_sp[:])
                        nc.sync.dma_start(
                            recr[0:1, :].rearrange("o (g i) -> o g i", g=128),
                            rec_sp[:],
                        )
                        evs.append((o_ev, recr))

                    def tail():
                        with nc.allow_low_precision(reason="fp32r"):
                            for h in range(2):
                                o_ev, recr = evs[h]
                                hb = h * 64
                                # 1/denom row -> 64 partitions on the idle
                                # GpSimd engine (frees the PE of the
                                # ones-matmul broadcast)
                                bc = sb_d.tile([64, IM], F32R, tag="bc")
                                nc.gpsimd.partition_broadcast(
                                    bc[:], recr[0:1, :]
                                )
                                nc.vector.tensor_mul(
                                    oT_b[hb : hb + 64, :],
                                    o_ev[0:64, :],
                                    bc[:],
                                )

                    tails.append(tail)
                    for ts in range(IM // 128):
                        projq.append(
                            (0.6, lambda ts=ts: proj_tile(ts, b, im, oT_b))
                        )

                pending_norm.append(norm_d)

                def proj_tile(ts, b=b, im=im, oT_b=oT_b):
                    y_t = sb_y.tile([128, D], mybir.dt.float32, tag="y")
                    for nh in range(2):
                        psC = ps_m.tile([128, 512], mybir.dt.float32, tag="m")
                        nc.tensor.matmul(
                            psC[:],
                            oT_b[:, ts * 128 : (ts + 1) * 128],
                            wo_t[:, nh * 512 : (nh + 1) * 512],
                            start=True,
                            stop=True,
                        )
                        if drain_mode["on"]:
                            # kernel drain: ACT is idle, let the scheduler
                            # spread evac copies across ACT+DVE instead of
                            # serializing the psC slot round-trip on DVE
                            nc.any.tensor_copy(
                                y_t[:, nh * 512 : (nh + 1) * 512], psC[:]
                            )
                        else:
                            nc.vector.tensor_copy(
                                y_t[:, nh * 512 : (nh + 1) * 512], psC[:]
                            )
                    t0 = b * L + im * IM + ts * 128
                    nc.sync.dma_start(y_d[t0 : t0 + 128, :], y_t[:])

        # batch 0: minimal eager stage A (q tm0 + k), rest through filler
        eager, fill0 = stage_a_units(0, split_first=True)
        for _c, u in eager:
            u()
        stagea.extend(fill0)
        for b in range(B):
            if b + 1 < B:
                stagea.extend(stage_a_units(b + 1)[1])
            # batch 0 drains its own residue early (v tiles are needed by
            # the first PV matmuls) plus batch 1's units; later batches
            # spread the next batch's stage A over their full span.
            pace["s_left"] = 40 if b == 0 else NIM * NJ
            stage_b(b)
        drain_mode["on"] = True
        for fn in pending_pv:
            fn()
        pending_pv.clear()
        for fn in pending_norm:
            fn()
        pending_norm.clear()
        while tails:
            tails.popleft()()
        while stagea:
            stagea.popleft()[1]()
        while projq:
            projq.popleft()[1]()
        while tails:
            tails.popleft()()
        assert not pending_norm and not stagea and not projq

    nc.compile()
    return nc


_PROGRAM = None
_LAST_EXEC_NS = None
_LAST_RESULT = None


def _get_program():
    global _PROGRAM
    if _PROGRAM is None:
        _PROGRAM = _build_program()
    return _PROGRAM


def kernel(x, mask, w_qkv, w_out):
    x = np.asarray(x)
    mask = np.asarray(mask)
    w_qkv = np.asarray(w_qkv)
    w_out = np.asarray(w_out)
    if not mask.all():
        return _masked_fallback(x, mask, w_qkv, w_out)

    from concourse.bass_utils import run_bass_kernel_spmd

    xT = _to_bf16(x.reshape(T, D).T)
    w4 = np.asarray(w_qkv, dtype=np.float32).reshape(D, 3, NH, HD)
    ones = np.ones((128, 64), dtype=np.float32)
    ident = np.eye(128, dtype=np.float32)
    in_maps = []
    for c in range(NCORES):
        hsel = [2 * c, 2 * c + 1]
        wc = _to_bf16(w4[:, :, hsel, :].reshape(D, 384))
        woc = _round_fp32r(w_out[2 * c * HD : (2 * c + 2) * HD, :])
        in_maps.append(
            {"xT": xT, "wqkv": wc, "wout": woc, "ones": ones, "ident": ident}
        )

    nc = _get_program()
    trace = os.environ.get("BASS_KERNEL_TRACE") == "1"
    res = run_bass_kernel_spmd(nc, in_maps, list(range(NCORES)), trace=trace)
    global _LAST_EXEC_NS, _LAST_RESULT
    _LAST_RESULT = res
    _LAST_EXEC_NS = getattr(res, "exec_time_ns", None)
    y = res.results[0]["y"].astype(np.float64)
    for c in range(1, NCORES):
        y += res.results[c]["y"]
    return y.astype(np.float32).reshape(B, L, D)


def _masked_fallback(x, mask, w_qkv, w_out):
    """Reference path for non-all-true masks (never hit for the spec inputs)."""
    b, l, d = x.shape
    scale = HD ** -0.5
    qkv = x.reshape(b * l, d) @ w_qkv
    qkv = qkv.reshape(b, l, 3, NH, HD).transpose(2, 0, 3, 1, 4)
    q, k, v = qkv[0], qkv[1], qkv[2]
    attn = np.einsum("bhnd,bhmd->bhnm", q, k) * scale
    attn = np.where(mask[:, None, :, :], attn, -np.inf)
    attn = attn - attn.max(axis=-1, keepdims=True)
    np.exp(attn, out=attn)
    attn /= attn.sum(axis=-1, keepdims=True)
    out = np.einsum("bhnm,bhmd->bhnd", attn, v)
    out = out.transpose(0, 2, 1, 3).reshape(b, l, d)
    return (out @ w_out).astype(np.float32)


# revision 53
# speedup vs baseline: 1.1323x; 1.0005x over previous
"""Multi-head attention (B=4, L=2048, D=1024, H=16, hd=64) on 8 Trainium2 cores.

Sharding: 8-way tensor parallel over heads. Core c owns heads (2c, 2c+1) for
all batches: it projects qkv for its heads (x replicated, w_qkv column-sliced),
runs attention, and computes a partial out-projection with its w_out row-slice.
The host sums the 8 partials (row-parallel unshard).

Final design notes (from NTFF trace analysis; 614us baseline -> ~495us):
- fp32r matmuls at N=512 already stream at ~225ns (1 col/cycle warm); bf16
  gains nothing on the PE and costs +220ns/exp on ACT writes. So matmul
  operands stay fp32r except stage A's x/w_qkv (bf16 halves the 32MB x DMA).
- The j-loop is gated by ScalarE exp ([128,1024] ACTIVATE = ~1.11us each,
  256 total = ~285us floor). Everything else must fit in its shadow.
- S^T matmuls contract over K=64 only. Processing both heads per j as
  adjacent matmuls at base partitions 0/64 puts them in distinct PE row
  groups, so they co-execute (~2x). Packing both heads' scores for a
  512-query block into one [128, 2, 512] PSUM tile keeps exp at N=1024
  while fitting PSUM: S 2bufs x 2banks + po 2x1 + scratch 2 = 8 banks.
- Softmax 1/denom: DVE reciprocal on a [1,512] row is ~4us (single lane).
  Instead DMA-spread the denominator row to [128,4] (4 elem/lane), DVE
  reciprocal there (~60ns), DMA-gather back to a row, then the usual
  ones-matmul partition broadcast + DVE multiply, all deferred off the
  exp critical path.
- HAM: the PE clock gate drops to 1.2GHz after ~3.4us of low activity; the
  baseline lost ~90us in its last batch (no filler left -> PE sparse ->
  half clock). Filler (stage A of later batches, out-projection, norm
  tails) is paced against the *global* remaining j-iterations so work
  remains to keep the PE dense through the final batch.
- Startup: eager stage A for batch 0 computes only q(tm0)+k(all) before the
  j-loop starts; v projection/transposes and q(tm1..3) drain as early
  filler. First exp issues ~25us earlier than with full eager stage A.
"""
import os
from collections import deque
import numpy as np
from contextlib import ExitStack

B, L, D = 4, 2048, 1024
NH, HD = 16, 64
T = B * L  # 8192 tokens
NCORES = 8
TM = 512  # stage-A token macro-tile
IM = 512  # stage-B query block


def _round_fp32r(a: np.ndarray) -> np.ndarray:
    """Round fp32 to fp32r (e8m11: fp32 with low 12 mantissa bits zero), RNE."""
    u = np.ascontiguousarray(a, dtype=np.float32).view(np.uint32).copy()
    add = np.uint32(0x7FF) + ((u >> np.uint32(12)) & np.uint32(1))
    u = (u + add) & np.uint32(0xFFFFF000)
    return u.view(np.float32)


def _to_bf16(a: np.ndarray):
    import ml_dtypes

    return np.asarray(a, dtype=np.float32).astype(ml_dtypes.bfloat16)


def _build_program():
    import concourse.bacc as bacc
    import concourse.tile as tile
    from concourse import mybir

    F32 = mybir.dt.float32
    F32R = mybir.dt.float32r
    BF16 = mybir.dt.bfloat16
    EXP = mybir.ActivationFunctionType.Exp

    nc = bacc.Bacc(
        "TRN2", target_bir_lowering=False, debug=False, num_devices=NCORES
    )
    xT_d = nc.dram_tensor("xT", [D, T], BF16, kind="ExternalInput")
    wqkv_d = nc.dram_tensor("wqkv", [D, 384], BF16, kind="ExternalInput")
    wout_d = nc.dram_tensor("wout", [128, D], F32R, kind="ExternalInput")
    ones_d = nc.dram_tensor("ones", [128, 64], F32R, kind="ExternalInput")
    ident_d = nc.dram_tensor("ident", [128, 128], F32R, kind="ExternalInput")
    y_d = nc.dram_tensor("y", [T, D], F32, kind="ExternalOutput")

    xT_v = xT_d.ap().rearrange("(k p) t -> p k t", p=128)  # [128, 8, T]
    wqkv_v = wqkv_d.ap().rearrange("(k p) c -> p k c", p=128)  # [128, 8, 384]

    NTM = L // TM  # stage-A macro tiles per batch
    NJ = L // 128  # key tiles per batch
    NIM = L // IM  # query blocks per batch

    with tile.TileContext(nc) as tc, ExitStack() as ctx:
        consts = ctx.enter_context(tc.tile_pool(name="consts", bufs=1))
        sb_x = ctx.enter_context(tc.tile_pool(name="sb_x", bufs=4))
        sb_qk = ctx.enter_context(tc.tile_pool(name="sb_qk", bufs=2))
        sb_v = ctx.enter_context(tc.tile_pool(name="sb_v", bufs=2))
        sb_vst = ctx.enter_context(tc.tile_pool(name="sb_vst", bufs=5))
        sb_p = ctx.enter_context(tc.tile_pool(name="sb_p", bufs=3))
        sb_o = ctx.enter_context(tc.tile_pool(name="sb_o", bufs=4))
        sb_d = ctx.enter_context(tc.tile_pool(name="sb_d", bufs=4))
        sb_oT = ctx.enter_context(tc.tile_pool(name="sb_oT", bufs=10))
        sb_y = ctx.enter_context(tc.tile_pool(name="sb_y", bufs=4))
        ps_s = ctx.enter_context(tc.tile_pool(name="ps_s", bufs=2, space="PSUM"))
        ps_po = ctx.enter_context(tc.tile_pool(name="ps_po", bufs=1, space="PSUM"))
        ps_m = ctx.enter_context(tc.tile_pool(name="ps_m", bufs=2, space="PSUM"))

        wq_t = consts.tile([128, 8, 384], BF16, tag="wqkv")
        nc.sync.dma_start(wq_t[:], wqkv_v[:])
        wo_t = consts.tile([128, D], F32R, tag="wout")
        nc.sync.dma_start(wo_t[:], wout_d[:])
        ones_t = consts.tile([128, 64], F32R, tag="ones")
        nc.sync.dma_start(ones_t[:], ones_d[:])
        ident_t = consts.tile([128, 128], F32R, tag="ident")
        nc.sync.dma_start(ident_t[:], ident_d[:])

        qk_tiles = {}  # b -> (qT, kT, v_aug)

        def stage_a_units(b, split_first=False):
            """Emitter closures for batch b's qkv projection.

            Returns (eager_units, filler_units). With split_first, the eager
            part is just q(tm0) + k(all tms) - the minimum for the first
            query block's S matmuls - and everything else goes to filler.
            """
            qT_b = sb_qk.tile([128, L], F32R, tag="qT")
            kT_b = sb_qk.tile([128, L], F32R, tag="kT")
            v_b = sb_v.tile([128, 2, NJ, 65], F32R, tag="v")
            qk_tiles[b] = (qT_b, kT_b, v_b)
            xt_tiles = {}
            vst_tiles = {}

            def ones_col():
                nc.vector.tensor_copy(
                    v_b[:, :, :, 64:65],
                    ones_t[:, 0 : 2 * NJ].rearrange(
                        "p (h j o) -> p h j o", h=2, o=1
                    ),
                )

            def xt_load(tm):
                if tm not in xt_tiles:
                    xt = sb_x.tile([128, 8, TM], BF16, tag="xt")
                    t0 = b * L + tm * TM
                    nc.sync.dma_start(xt[:], xT_v[:, :, t0 : t0 + TM])
                    xt_tiles[tm] = xt

            def col_group(tm, c):
                xt_load(tm)
                xt = xt_tiles[tm]
                psA = ps_m.tile([128, TM], mybir.dt.float32, tag="m")
                for k in range(8):
                    nc.tensor.matmul(
                        psA[:],
                        wq_t[:, k, c * 128 : (c + 1) * 128],
                        xt[:, k, :],
                        start=(k == 0),
                        stop=(k == 7),
                    )
                if c == 0:
                    nc.vector.tensor_copy(qT_b[:, tm * TM : (tm + 1) * TM], psA[:])
                elif c == 1:
                    nc.vector.tensor_copy(kT_b[:, tm * TM : (tm + 1) * TM], psA[:])
                else:
                    vst = sb_vst.tile([128, TM], F32R, tag="vst")
                    nc.vector.tensor_copy(vst[:], psA[:])
                    vst_tiles[tm] = vst

            def transposes(tm):
                vst = vst_tiles[tm]
                for tb in range(TM // 128):
                    jt = tm * (TM // 128) + tb
                    ptr = ps_m.tile([128, 128], F32R, tag="m")
                    nc.tensor.transpose(
                        ptr[:], vst[:, tb * 128 : (tb + 1) * 128], ident_t[:]
                    )
                    nc.vector.tensor_copy(v_b[:, 0, jt, 0:64], ptr[:, 0:64])
                    nc.vector.tensor_copy(v_b[:, 1, jt, 0:64], ptr[:, 64:128])

            if split_first:
                # eager: everything the first PV iteration needs (q tm0,
                # all k, v tm0); the rest carries a hard deadline (global
                # j-iteration index) enforced by pop_filler, because Tile
                # preserves emission order - emitting a consumer before its
                # producer is a silent stale read, not a stall.
                # eager: everything the first PV iteration needs (q tm0,
                # all k, v tm0); the rest carries a hard deadline (global
                # j-iteration index) enforced by pop_filler, because Tile
                # preserves emission order - emitting a consumer before its
                # producer is a silent stale read, not a stall.
                eager = [(1.0, lambda: col_group(0, 0))]
                eager += [
                    (0.0, lambda tm=tm: xt_load(tm)) for tm in range(1, NTM)
                ]
                eager += [
                    (1.0, lambda tm=tm: col_group(tm, 1)) for tm in range(NTM)
                ]
                eager.append((1.0, lambda: col_group(0, 2)))
                eager.append((0.7, lambda: transposes(0)))
                eager.append((0.1, ones_col))
                fill = []
                for tm in range(1, NTM):
                    # v/tr(tm) feed PV j=4*tm (emitted at iter 4*tm+1);
                    # q(tm) feeds S of im=tm (emitted at iter 16*tm).
                    fill.append(
                        (1.0, lambda tm=tm: col_group(tm, 2), 4 * tm - 1)
                    )
                    fill.append((0.7, lambda tm=tm: transposes(tm), 4 * tm))
                    fill.append(
                        (1.0, lambda tm=tm: col_group(tm, 0), 16 * tm - 2)
                    )
                return eager, fill
            fill = [(0.1, ones_col, None), (0.0, lambda: xt_load(0), None)]
            for tm in range(NTM):
                # prefetch the next tm's x tile so its DMA overlaps this
                # tm's matmuls instead of stalling them
                if tm + 1 < NTM:
                    fill.append((0.0, lambda tm=tm: xt_load(tm + 1), None))
                for c in range(3):
                    fill.append(
                        (1.0, lambda tm=tm, c=c: col_group(tm, c), None)
                    )
                fill.append((0.7, lambda tm=tm: transposes(tm), None))
            return [], fill

        # Three filler queues with different pacing/deadlines:
        # - tails: norm broadcast+multiply; run promptly (~1/iter) because
        #   they release pool slots (o_ev/recr/pbc) that in-order engines
        #   would otherwise deadlock on.
        # - stagea: next batch's qkv projection; must finish by its batch
        #   start, paced over the current batch's iterations.
        # - projq: out-projection tiles; paced over the *global* remaining
        #   iterations so PE work remains through the final batch (HAM).
        tails = deque()
        stagea = deque()
        projq = deque()

        drain_mode = {"on": False}
        pending_norm = []
        pending_pv = []  # final PV pair of an im, emitted after the next
        # im's S pair so the next exp is not queued behind it on the PE
        pace = {
            "s_credit": 0.0,
            "s_left": 1,
            "p_credit": 0.0,
            # finish the proj queue ~12 iterations early so the kernel
            # drain (which runs PE-sparse and HAM-throttled) stays short
            "g_left": B * NIM * NJ - 12,
            "it": 0,  # global j-iteration counter (deadline clock)
        }

        def pop_filler():
            if tails:
                tails.popleft()()
            total = sum(c for c, _, _ in stagea)
            rate = total / max(pace["s_left"], 1)
            pace["s_credit"] += rate
            while stagea and (
                pace["s_credit"] >= stagea[0][0] * 0.5
                or (stagea[0][2] is not None and stagea[0][2] <= pace["it"])
            ):
                c, fn, _dl = stagea.popleft()
                pace["s_credit"] = max(pace["s_credit"] - c, 0.0)
                fn()
            pace["s_left"] = max(pace["s_left"] - 1, 1)
            total = sum(c for c, _ in projq)
            rate = total / max(pace["g_left"], 1)
            pace["p_credit"] += rate
            while projq and pace["p_credit"] >= projq[0][0] * 0.5:
                c, fn = projq.popleft()
                pace["p_credit"] -= c
                fn()
            # floor at 6 so the horizon never cliff-drains the whole queue
            # in one iteration; stragglers finish in the kernel drain
            pace["g_left"] = max(pace["g_left"] - 1, 6)
            pace["it"] += 1

        def stage_b(b):
            qT_b, kT_b, v_b = qk_tiles[b]
            for im in range(NIM):
                i0 = im * IM
                oT_b = sb_oT.tile([128, IM], F32R, tag="oT")
                po = [
                    ps_po.tile(
                        [65, IM], mybir.dt.float32, tag=f"po{h}", name=f"po{h}"
                    )
                    for h in range(2)
                ]
                p_prev = None
                for j in range(NJ):
                    ps = ps_s.tile([128, 2, IM], mybir.dt.float32, tag="s")
                    for h in range(2):
                        hb = h * 64
                        nc.tensor.matmul(
                            ps[:, h, :],
                            kT_b[hb : hb + 64, j * 128 : (j + 1) * 128],
                            qT_b[hb : hb + 64, i0 : i0 + IM],
                            start=True,
                            stop=True,
                        )
                    if j == 0:
                        for fn in pending_pv:
                            fn()
                        pending_pv.clear()
                        for fn in pending_norm:
                            fn()
                        pending_norm.clear()
                    p_t = sb_p.tile([128, 2, IM], F32R, tag="p")
                    nc.scalar.activation(p_t[:], ps[:], EXP, scale=0.125)
                    if p_prev is not None:
                        jp, pp = p_prev
                        for h in range(2):
                            nc.tensor.matmul(
                                po[h][:],
                                v_b[:, h, jp, :],
                                pp[:, h, :],
                                start=(jp == 0),
                                stop=False,
                            )
                    p_prev = (j, p_t)
                    pop_filler()
                jp, pp = p_prev

                def pv_final(po=po, v_b=v_b, jp=jp, pp=pp):
                    for h in range(2):
                        nc.tensor.matmul(
                            po[h][:],
                            v_b[:, h, jp, :],
                            pp[:, h, :],
                            start=False,
                            stop=True,
                        )

                pending_pv.append(pv_final)

                def norm_d(po=po, oT_b=oT_b, im=im, b=b):
                    # Fast part at flush: evacuate po (releases the PSUM
                    # bank) and start the 1/denom chain. The denominator
                    # row is DMA-spread across partitions so the DVE
                    # reciprocal runs ~60ns instead of ~4us single-lane.
                    evs = []
                    for h in range(2):
                        o_ev = sb_o.tile([65, IM], mybir.dt.float32, tag="oe")
                        den_sp = sb_d.tile(
                            [128, IM // 128], mybir.dt.float32, tag="dsp"
                        )
                        rec_sp = sb_d.tile([128, IM // 128], F32R, tag="rsp")
                        recr = sb_d.tile([65, IM], F32R, tag="recr")
                        nc.vector.tensor_copy(o_ev[:], po[h][:])
                        nc.sync.dma_start(
                            den_sp[:],
                            o_ev[64:65, :].rearrange("o (g i) -> o g i", g=128),
                        )
                        with nc.allow_low_precision(reason="fp32r recip"):
                            nc.vector.reciprocal(rec_sp[:], den_sp[:])
                        nc.sync.dma_start(
                            recr[0:1, :].rearrange("o (g i) -> o g i", g=128),
                            rec_sp[:],
                        )
                        evs.append((o_ev, recr))

                    def tail():
                        with nc.allow_low_precision(reason="fp32r"):
                            for h in range(2):
                                o_ev, recr = evs[h]
                                hb = h * 64
                                # 1/denom row -> 64 partitions on the idle
                                # GpSimd engine (frees the PE of the
                                # ones-matmul broadcast)
                                bc = sb_d.tile([64, IM], F32R, tag="bc")
                                nc.gpsimd.partition_broadcast(
                                    bc[:], recr[0:1, :]
                                )
                                nc.vector.tensor_mul(
                                    oT_b[hb : hb + 64, :],
                                    o_ev[0:64, :],
                                    bc[:],
                                )

                    tails.append(tail)
                    for ts in range(IM // 128):
                        projq.append(
                            (0.6, lambda ts=ts: proj_tile(ts, b, im, oT_b))
                        )

                pending_norm.append(norm_d)

                def proj_tile(ts, b=b, im=im, oT_b=oT_b):
                    y_t = sb_y.tile([128, D], mybir.dt.float32, tag="y")
                    for nh in range(2):
                        psC = ps_m.tile([128, 512], mybir.dt.float32, tag="m")
                        nc.tensor.matmul(
                            psC[:],
                            oT_b[:, ts * 128 : (ts + 1) * 128],
                            wo_t[:, nh * 512 : (nh + 1) * 512],
                            start=True,
                            stop=True,
                        )
                        if drain_mode["on"]:
                            # kernel drain: ACT is idle, let the scheduler
                            # spread evac copies across ACT+DVE instead of
                            # serializing the psC slot round-trip on DVE
                            nc.any.tensor_copy(
                                y_t[:, nh * 512 : (nh + 1) * 512], psC[:]
                            )
                        else:
                            nc.vector.tensor_copy(
                                y_t[:, nh * 512 : (nh + 1) * 512], psC[:]
                            )
                    t0 = b * L + im * IM + ts * 128
                    nc.sync.dma_start(y_d[t0 : t0 + 128, :], y_t[:])

        # batch 0: minimal eager stage A (q tm0 + k), rest through filler
        eager, fill0 = stage_a_units(0, split_first=True)
        for _c, u in eager:
            u()
        stagea.extend(fill0)
        for b in range(B):
            if b + 1 < B:
                stagea.extend(stage_a_units(b + 1)[1])
            # batch 0's residue drains via its hard deadlines; every
            # batch spreads the next batch's stage A over its full span.
            pace["s_left"] = NIM * NJ
            stage_b(b)
        drain_mode["on"] = True
        for fn in pending_pv:
            fn()
        pending_pv.clear()
        for fn in pending_norm:
            fn()
        pending_norm.clear()
        while tails:
            tails.popleft()()
        while stagea:
            stagea.popleft()[1]()
        while projq:
            projq.popleft()[1]()
        while tails:
            tails.popleft()()
        assert not pending_norm and not stagea and not projq

    nc.compile()
    return nc


_PROGRAM = None
_LAST_EXEC_NS = None
_LAST_RESULT = None


def _get_program():
    global _PROGRAM
    if _PROGRAM is None:
        _PROGRAM = _build_program()
    return _PROGRAM


def kernel(x, mask, w_qkv, w_out):
    x = np.asarray(x)
    mask = np.asarray(mask)
    w_qkv = np.asarray(w_qkv)
    w_out = np.asarray(w_out)
    if not mask.all():
        return _masked_fallback(x, mask, w_qkv, w_out)

    from concourse.bass_utils import run_bass_kernel_spmd

    xT = _to_bf16(x.reshape(T, D).T)
    w4 = np.asarray(w_qkv, dtype=np.float32).reshape(D, 3, NH, HD)
    ones = np.ones((128, 64), dtype=np.float32)
    ident = np.eye(128, dtype=np.float32)
    in_maps = []
    for c in range(NCORES):
        hsel = [2 * c, 2 * c + 1]
        wc = _to_bf16(w4[:, :, hsel, :].reshape(D, 384))
        woc = _round_fp32r(w_out[2 * c * HD : (2 * c + 2) * HD, :])
        in_maps.append(
            {"xT": xT, "wqkv": wc, "wout": woc, "ones": ones, "ident": ident}
        )

    nc = _get_program()
    trace = os.environ.get("BASS_KERNEL_TRACE") == "1"
    res = run_bass_kernel_spmd(nc, in_maps, list(range(NCORES)), trace=trace)
    global _LAST_EXEC_NS, _LAST_RESULT
    _LAST_RESULT = res
    _LAST_EXEC_NS = getattr(res, "exec_time_ns", None)
    y = res.results[0]["y"].astype(np.float64)
    for c in range(1, NCORES):
        y += res.results[c]["y"]
    return y.astype(np.float32).reshape(B, L, D)


def _masked_fallback(x, mask, w_qkv, w_out):
    """Reference path for non-all-true masks (never hit for the spec inputs)."""
    b, l, d = x.shape
    scale = HD ** -0.5
    qkv = x.reshape(b * l, d) @ w_qkv
    qkv = qkv.reshape(b, l, 3, NH, HD).transpose(2, 0, 3, 1, 4)
    q, k, v = qkv[0], qkv[1], qkv[2]
    attn = np.einsum("bhnd,bhmd->bhnm", q, k) * scale
    attn = np.where(mask[:, None, :, :], attn, -np.inf)
    attn = attn - attn.max(axis=-1, keepdims=True)
    np.exp(attn, out=attn)
    attn /= attn.sum(axis=-1, keepdims=True)
    out = np.einsum("bhnm,bhmd->bhnd", attn, v)
    out = out.transpose(0, 2, 1, 3).reshape(b, l, d)
    return (out @ w_out).astype(np.float32)
